# revision 1
# baseline (speedup 1.0000x reference)
"""LSTM encoder (B=64, S=512, E=H=1024) on 8 trn2 NeuronCores.

Strategy:
  - Tensor-parallel over the 4H gate dimension: each core owns 128 hidden
    channels (x4 gates = 512 gate rows), the full batch (64), and the full
    sequence.
  - Phase 1 (parallel): embedding gather via dma_gather(transpose=True)
    directly into X^T layout, then gx = W_ih_local @ X^T for all 32768
    tokens, stored to DRAM as bf16.
  - Phase 2 (recurrence): 512 sequential steps. Per step: gates.T =
    identity-matmul(gx_t) + sum_k W_hh_chunk @ h_chunk (PSUM accumulate),
    sigmoid/tanh on ScalarE, cell update on VectorE, then the new local
    h slice (128 ch x 64 batch, bf16) is pushed to all 7 peers' SBUF via
    remote_dma_broadcast (XOR-relative addressing, so the same SPMD
    program works on every core; per-core W_hh is XOR-permuted on host).
  - h lives in [-1,1]: bf16 exchange; c stays fp32 on-core.

Self-contained: hardcodes all shapes; host-side prep is numpy only.
"""

import os
import sys

sys.path.insert(0, "/opt/trn_rl_repo")

import numpy as np
import ml_dtypes

import concourse.bass as bass
import concourse.bacc as bacc
import concourse.mybir as mybir

BF16 = ml_dtypes.bfloat16
AF = mybir.ActivationFunctionType
dt = mybir.dt

# problem constants
VOCAB, EMB, HID = 32000, 1024, 1024
B = 64
S = 512
CORES = 8
KC = 8            # contraction chunks of 128
NCHUNK = 4        # gate chunks per core (order: g, i, f, o)
G = NCHUNK * 128  # 512 gate rows per core
NT = 512          # tokens per phase-1 tile
TPT = NT // B     # timesteps per phase-1 tile (8)
# pytorch gate blocks in W rows: i, f, g, o ; our chunk order: g, i, f, o
CHUNK_TO_BLOCK = [2, 0, 1, 3]

# Logical(replica) -> physical TPB mapping on trn2 (driver V0 table, the
# per-chip base offset cancels): upper-die pairs are swapped. The remote
# broadcast's relative (0, d) dest goes to physical (p ^ d), so replica r's
# slot d receives from replica PERM[r][d] = m(m(r) ^ d). HW-verified by a
# probe kernel (each core broadcast its id; see T table in dev notes).
_M = [0, 1, 2, 3, 6, 7, 4, 5]


def sender_at_slot(r, d):
    return _M[_M[r] ^ d]


def build(nc_steps=S, exchange="remote", nbcast=7, wait_rsem=True):
    """Emit the SPMD bass program (identical on all 8 cores)."""
    nsteps = nc_steps
    TT = B * nsteps // NT  # number of phase-1 token tiles
    assert B * nsteps % NT == 0

    nc = bacc.Bacc(None, target_bir_lowering=False)

    # ---- kernel I/O (per core) ----
    emb_d = nc.declare_dram_parameter("emb16", [VOCAB, EMB], dt.bfloat16, isOutput=False)
    idx_d = nc.declare_dram_parameter("idx", [TT, 128, NT // 16], dt.int16, isOutput=False)
    wih_d = nc.declare_dram_parameter("w_ih", [128, KC * G], dt.bfloat16, isOutput=False)
    whh_d = nc.declare_dram_parameter("w_hh", [128, KC * G], dt.bfloat16, isOutput=False)
    ident_d = nc.declare_dram_parameter("ident", [128, 128], dt.bfloat16, isOutput=False)
    gbias_d = nc.declare_dram_parameter("gbias", [128, NCHUNK], dt.float32, isOutput=False)
    out_d = nc.declare_dram_parameter("out", [2, 128, B], dt.float32, isOutput=True)

    # ---- DRAM scratch ----
    gx_d = nc.dram_tensor("gx", [128, nsteps, NCHUNK * B], dt.bfloat16)
    bar_in = nc.dram_tensor("bar_in", [128, 4], dt.float32)
    bar_out = nc.dram_tensor("bar_out", [128, 4], dt.float32, addr_space="Shared")

    # ---- semaphores ----
    cc_sem = nc.alloc_semaphore("cc_sem")
    bar_sem = nc.alloc_semaphore("bar_sem")
    bardma_sem = nc.alloc_semaphore("bardma_sem")
    wload = nc.alloc_semaphore("wload")
    g_sem = [nc.alloc_semaphore("g_sem0"), nc.alloc_semaphore("g_sem1")]
    mm1 = nc.alloc_semaphore("mm1")
    cp_sem = nc.alloc_semaphore("cp_sem")
    st_sem = [nc.alloc_semaphore("st_sem0"), nc.alloc_semaphore("st_sem1")]
    gxd = [nc.alloc_semaphore("gxd0"), nc.alloc_semaphore("gxd1")]
    idm = nc.alloc_semaphore("idm")
    mmr = nc.alloc_semaphore("mmr")
    act_s = nc.alloc_semaphore("act_s")
    dve_s = nc.alloc_semaphore("dve_s")
    prep_s = nc.alloc_semaphore("prep_s")
    # parity-split: exchange e increments index (e+1)%2; the 2-step pipeline
    # separation guarantees no cross-exchange mixing within one parity chain.
    rsem = [nc.alloc_semaphore("rsem0"), nc.alloc_semaphore("rsem1")]  # +2 x7 per exchange
    lsem = [nc.alloc_semaphore("lsem0"), nc.alloc_semaphore("lsem1")]  # +16 x7 per exchange
    fin = nc.alloc_semaphore("fin")

    from contextlib import ExitStack

    with ExitStack() as ctx:
        sb = lambda name, shape, d: ctx.enter_context(nc.sbuf_tensor(name, shape, d))
        idx_sb = sb("idx_sb", [128, TT * (NT // 16)], dt.int16)
        wih_sb = sb("wih_sb", [128, KC * G], dt.bfloat16)
        whh_sb = sb("whh_sb", [128, KC * G], dt.bfloat16)
        ident_sb = sb("ident_sb", [128, 128], dt.bfloat16)
        gbias_sb = sb("gbias_sb", [128, NCHUNK], dt.float32)
        xt = [sb(f"xt{i}", [128, KC, NT], dt.bfloat16) for i in range(2)]
        stage = [sb(f"stage{i}", [128, TPT * NCHUNK * B], dt.bfloat16) for i in range(2)]
        hg = [sb(f"hg{i}", [128, CORES * B], dt.bfloat16) for i in range(2)]
        gxt = [sb(f"gxt{i}", [128, NCHUNK * B], dt.bfloat16) for i in range(2)]
        sg = sb("sg", [128, NCHUNK * B], dt.float32)
        ig_sb = sb("ig_sb", [128, B], dt.float32)
        fc_sb = sb("fc_sb", [128, B], dt.float32)
        thc_sb = sb("thc_sb", [128, B], dt.float32)
        c_sb = sb("c_sb", [128, B], dt.float32)
        hout_sb = sb("hout_sb", [128, B], dt.float32)
        bar_sb = sb("bar_sb", [128, 4], dt.float32)
        # PSUM: 8 tensors of [128, 512] fp32 = 8 full banks
        psum = [
            ctx.enter_context(nc.psum_tensor(f"ps{i}", [128, 512], dt.float32))
            for i in range(8)
        ]
        block = ctx.enter_context(nc.Block())

        NIDX = NT // 16  # idx columns per tile

        # =========== SYNC engine: weight loads, phase-1 stores, ===========
        # =========== phase-2 gx prefetch, final output            ===========
        @block.sync
        def _(sy):
            # preload constants (HWDGE, FIFO order)
            sy.dma_start(
                out=idx_sb.ap().rearrange("p (t c) -> p t c", t=TT),
                in_=idx_d.ap().rearrange("t p c -> p t c"),
            ).then_inc(wload, 16)
            sy.dma_start(out=wih_sb[:, :], in_=wih_d[:, :]).then_inc(wload, 16)
            sy.dma_start(out=whh_sb[:, :], in_=whh_d[:, :]).then_inc(wload, 16)
            sy.dma_start(out=ident_sb[:, :], in_=ident_d[:, :]).then_inc(wload, 16)
            sy.dma_start(out=gbias_sb[:, :], in_=gbias_d[:, :]).then_inc(wload, 16)

            # phase-1 stores
            for tau in range(TT):
                sy.wait_ge(cp_sem, 4 * tau + 4)
                sy.dma_start(
                    out=gx_d[:, TPT * tau : TPT * (tau + 1), :],
                    in_=stage[tau % 2].ap().rearrange("p (t e) -> p t e", t=TPT),
                ).then_inc(st_sem[tau % 2], 16)

            # phase-2 gx prefetch: first two, then rolling
            sy.dma_start(out=gxt[0][:, :], in_=gx_d[:, 0, :]).then_inc(gxd[0], 16)
            if nsteps > 1:
                sy.dma_start(out=gxt[1][:, :], in_=gx_d[:, 1, :]).then_inc(gxd[1], 16)
            for t in range(2, nsteps):
                sy.wait_ge(idm, t - 1)
                sy.dma_start(out=gxt[t % 2][:, :], in_=gx_d[:, t, :]).then_inc(gxd[t % 2], 16)

            # final outputs
            sy.wait_ge(dve_s, 1 + 4 * nsteps)
            sy.dma_start(out=out_d[0, :, :], in_=hout_sb[:, :]).then_inc(fin, 16)
            sy.dma_start(out=out_d[1, :, :], in_=c_sb[:, :]).then_inc(fin, 16)
            sy.wait_ge(fin, 32)

        # =========== GPSIMD: barrier, gathers, h broadcast ===========
        @block.gpsimd
        def _(gp):
            # cross-core barrier: protects remote-sem increments from
            # racing a peer's kernel-start semaphore init.
            gp.memset(bar_sb[:, :], 0.0).then_inc(bar_sem, 1)
            gp.wait_ge(bar_sem, 1)
            gp.dma_start(out=bar_in[:, :], in_=bar_sb[:, :]).then_inc(bardma_sem, 16)
            gp.wait_ge(bardma_sem, 16)
            gp.collective_compute(
                "AllReduce",
                mybir.AluOpType.add,
                ins=[bar_in.ap().opt()],
                outs=[bar_out.ap().opt()],
                replica_groups=[list(range(CORES))],
            ).then_inc(cc_sem, 1)

            # phase-1 embedding gathers (transposing: out[p, k, j] = emb[idx_j, 128k+p])
            gp.wait_ge(wload, 80)  # constants loaded (incl. idx_sb)
            for tau in range(TT):
                if tau >= 2:
                    gp.wait_ge(mm1, 4 * (tau - 2) + 4)  # xt buffer free
                gp.dma_gather(
                    out_ap=xt[tau % 2][:, :, :],
                    in_ap=emb_d[:, :],
                    idxs_ap=idx_sb[:, NIDX * tau : NIDX * (tau + 1)],
                    num_idxs=NT,
                    num_idxs_reg=NT,
                    elem_size=EMB,
                    transpose=True,
                ).then_inc(g_sem[tau % 2], 16)

            # phase-2 h exchange: 7 broadcast preps + 1 trigger per step
            if exchange == "remote":
                gp.wait_ge(cc_sem, 1)
                for t in range(nsteps - 1):
                    po = (t + 1) % 2  # parity of the buffer holding h(t)
                    for d in range(1, 1 + nbcast):
                        rd = [None] * CORES
                        rd[d] = (0, d)
                        gp.remote_dma_broadcast(
                            out_ap=hg[po][:, B * d : B * (d + 1)],
                            in_ap=hg[po][:, 0:B],
                            remote_sem=rsem[po],
                            local_sem=lsem[po],
                            rdests=rd,
                        ).then_inc(prep_s, 1)
                    gp.wait_ge(prep_s, nbcast * (t + 1))
                    gp.wait_ge(dve_s, 1 + 4 * t + 4)  # h(t) written
                    gp.trigger_dma(count=nbcast)

        # =========== TENSOR engine ===========
        @block.tensor
        def _(te):
            te.wait_ge(wload, 80)
            # ---- phase 1 ----
            for tau in range(TT):
                te.wait_ge(g_sem[tau % 2], 16 * (tau // 2 + 1))
                for cb in range(NCHUNK):
                    pb = psum[(tau % 2) * 4 + cb]
                    if tau >= 2:
                        te.wait_ge(cp_sem, 4 * (tau - 2) + cb + 1)
                    for k in range(KC):
                        mm = te.matmul(
                            pb[:, :],
                            lhsT=wih_sb[:, G * k + 128 * cb : G * k + 128 * (cb + 1)],
                            rhs=xt[tau % 2][:, k, :],
                            start=(k == 0),
                            stop=(k == KC - 1),
                        )
                    mm.then_inc(mm1, 1)

            # ---- phase 2 ----
            for t in range(nsteps):
                P = t % 2
                # identity-matmul loads gx_t into psum (one per gate bank)
                # (first two prefetches are unordered w.r.t. each other)
                te.wait_ge(gxd[t % 2], 16 * (t // 2 + 1))
                if t < 2:
                    te.wait_ge(cp_sem, 4 * TT)  # phase-1 copies fully drained
                else:
                    te.wait_ge(act_s, 5 * (t - 2) + 4)  # psum parity reuse
                for cb in range(NCHUNK):
                    mm = te.matmul(
                        psum[P * 4 + cb][:, 0:B],
                        lhsT=ident_sb[:, :],
                        rhs=gxt[P][:, B * cb : B * (cb + 1)],
                        start=True,
                        stop=(t == 0),
                    )
                    if cb == NCHUNK - 1:
                        mm.then_inc(idm, 1)
                if t >= 1:
                    te.wait_ge(dve_s, 1 + 4 * t)  # own h slice in hg[P][:, 0:B]
                    if exchange == "remote" and wait_rsem:
                        te.wait_ge(rsem[t % 2], 2 * nbcast * ((t + 1) // 2))
                    for cb in range(NCHUNK):
                        for d in range(CORES):
                            mm = te.matmul(
                                psum[P * 4 + cb][:, 0:B],
                                lhsT=whh_sb[:, G * d + 128 * cb : G * d + 128 * (cb + 1)],
                                rhs=hg[P][:, B * d : B * (d + 1)],
                                start=False,
                                stop=(d == CORES - 1),
                            )
                        mm.then_inc(mmr, 1)

        # =========== SCALAR engine (ACT) ===========
        @block.scalar
        def _(sc):
            sc.wait_ge(wload, 80)
            # ---- phase 1: psum -> stage (bf16 cast) ----
            for tau in range(TT):
                for cb in range(NCHUNK):
                    sc.wait_ge(mm1, 4 * tau + cb + 1)
                    if tau >= 2:
                        sc.wait_ge(st_sem[tau % 2], 16 * (tau // 2))  # stage free
                    src = psum[(tau % 2) * 4 + cb].ap().rearrange("p (t b) -> p t b", t=TPT)
                    dst = stage[tau % 2].ap().rearrange(
                        "p (t e b) -> p t e b", t=TPT, e=NCHUNK
                    )[:, :, cb, :]
                    sc.activation(dst, src, AF.Copy).then_inc(cp_sem, 1)

            # ---- phase 2 activations ----
            # chunk order: 0=g(tanh), 1=i, 2=f, 3=o (sigmoid); then tanh(c)
            for t in range(nsteps):
                P = t % 2
                for cb in range(NCHUNK):
                    if t == 0:
                        sc.wait_ge(idm, 1)
                    else:
                        sc.wait_ge(mmr, 4 * (t - 1) + cb + 1)
                    fn = AF.Tanh if cb == 0 else AF.Sigmoid
                    sc.activation(
                        sg[:, B * cb : B * (cb + 1)],
                        psum[P * 4 + cb][:, 0:B],
                        fn,
                        bias=gbias_sb[:, cb : cb + 1],
                    ).then_inc(act_s, 1)
                sc.wait_ge(dve_s, 1 + 4 * t + 3)  # c updated
                sc.activation(thc_sb[:, :], c_sb[:, :], AF.Tanh).then_inc(act_s, 1)

        # =========== VECTOR engine (DVE) ===========
        @block.vector
        def _(ve):
            ve.memset(c_sb[:, :], 0.0).then_inc(dve_s, 1)
            for t in range(nsteps):
                Pn = (t + 1) % 2
                ve.wait_ge(act_s, 5 * t + 2)
                ve.tensor_mul(ig_sb[:, :], sg[:, B : 2 * B], sg[:, 0:B]).then_inc(dve_s, 1)
                ve.wait_ge(act_s, 5 * t + 3)
                # c_sb RAW from previous step's update (or the memset)
                ve.wait_ge(dve_s, max(1, 1 + 4 * (t - 1) + 3))
                ve.tensor_mul(fc_sb[:, :], sg[:, 2 * B : 3 * B], c_sb[:, :]).then_inc(dve_s, 1)
                ve.wait_ge(dve_s, 1 + 4 * t + 2)  # ig, fc writebacks landed
                ve.tensor_add(c_sb[:, :], ig_sb[:, :], fc_sb[:, :]).then_inc(dve_s, 1)
                ve.wait_ge(act_s, 5 * t + 5)
                if t == nsteps - 1:
                    ve.tensor_mul(hout_sb[:, :], sg[:, 3 * B : 4 * B], thc_sb[:, :]).then_inc(dve_s, 1)
                else:
                    if t >= 2 and exchange == "remote":
                        ve.wait_ge(lsem[(t + 1) % 2], 16 * nbcast * (t // 2))
                    ve.tensor_mul(hg[Pn][:, 0:B], sg[:, 3 * B : 4 * B], thc_sb[:, :]).then_inc(dve_s, 1)

    nc.compile()
    return nc


# ---------------------------------------------------------------------------
# host-side input prep
# ---------------------------------------------------------------------------

def prepare_in_maps(source, emb, W_ih, W_hh, b_ih, b_hh, nsteps=S):
    source = np.asarray(source)
    emb = np.asarray(emb, np.float32)
    W_ih = np.asarray(W_ih, np.float32)
    W_hh = np.asarray(W_hh, np.float32)
    b = np.asarray(b_ih, np.float32) + np.asarray(b_hh, np.float32)

    TT = B * nsteps // NT
    emb16 = emb.astype(BF16)
    ident = np.eye(128, dtype=BF16)

    # indices, wrapped: idx[tau, p, s] = source[b, TPT*tau + t'] with
    # j = s*16 + (p % 16), t' = j // 64, b = j % 64
    idx = np.zeros([TT, 128, NT // 16], np.int16)
    j = np.arange(NT)
    tprime, bb = j // B, j % B
    for tau in range(TT):
        ids = source[bb, TPT * tau + tprime].astype(np.int16)  # [NT]
        wrapped = ids.reshape(NT // 16, 16).T  # [16, NT//16]
        idx[tau] = np.tile(wrapped, (8, 1))

    in_maps = []
    H = HID
    for j_core in range(CORES):
        rows = np.concatenate(
            [
                np.arange(CHUNK_TO_BLOCK[cb] * H + 128 * j_core,
                          CHUNK_TO_BLOCK[cb] * H + 128 * (j_core + 1))
                for cb in range(NCHUNK)
            ]
        )
        Wi = W_ih[rows]  # [512, 1024]
        Wh = W_hh[rows]
        bi = b[rows]  # [512]

        # w_ih[p, G*k + 128*cb + m] = Wi[128*cb + m, 128*k + p]
        wi4 = Wi.reshape(NCHUNK, 128, KC, 128)          # [cb, m, k, p]
        wih = np.transpose(wi4, (3, 2, 0, 1)).reshape(128, KC * G).astype(BF16)

        # w_hh with XOR-permuted k chunks: position d holds chunk (j_core ^ d)
        wh4 = Wh.reshape(NCHUNK, 128, KC, 128)          # [cb, m, k, p]
        wh4p = wh4[:, :, [sender_at_slot(j_core, d) for d in range(KC)], :]
        whh = np.transpose(wh4p, (3, 2, 0, 1)).reshape(128, KC * G).astype(BF16)

        gbias = bi.reshape(NCHUNK, 128).T.copy().astype(np.float32)  # [128, 4]

        in_maps.append(
            {
                "emb16": emb16,
                "idx": idx,
                "w_ih": wih,
                "w_hh": whh,
                "ident": ident,
                "gbias": gbias,
            }
        )
    return in_maps


_BUILD_CACHE = {}


def _get_nc(nsteps=S, exchange="remote"):
    key = (nsteps, exchange)
    if key not in _BUILD_CACHE:
        _BUILD_CACHE[key] = build(nsteps, exchange)
    return _BUILD_CACHE[key]


def kernel(source, emb, W_ih, W_hh, b_ih, b_hh, _trace=False):
    from concourse.bass_utils import run_bass_kernel_spmd

    nc = _get_nc()
    in_maps = prepare_in_maps(source, emb, W_ih, W_hh, b_ih, b_hh)
    res = run_bass_kernel_spmd(nc, in_maps, core_ids=list(range(CORES)), trace=_trace)
    outs = [res.results[i]["out"] for i in range(CORES)]  # each [2, 128, B]
    h = np.concatenate([o[0].T for o in outs], axis=1)  # [B, 8*128]
    c = np.concatenate([o[1].T for o in outs], axis=1)
    out = np.stack([h, c]).astype(np.float32)
    if _trace:
        return out, res
    return out


# ---------------------------------------------------------------------------
# dev: multi-core simulation on a reduced problem
# ---------------------------------------------------------------------------

def _simulate(nsteps=8, exchange="remote", check_with_hw=False):
    from concourse import bass_interp, libnrt

    # no /dev/neuron on the axon client: fake the driver's logical->physical
    # NC map with the standard trn2 XOR-4 die-flip table (any XOR-affine
    # bijection preserves the kernel's XOR-relative addressing scheme).
    libnrt.get_trn2_nc_mapping.cache_clear()
    libnrt.nc_to_real_nc.cache_clear()
    libnrt.get_trn2_nc_mapping.__wrapped__.__globals__  # noqa: ensure attr exists
    fake_map = {(d, i): _M[i] for d in range(16) for i in range(8)}
    libnrt.get_trn2_nc_mapping = lambda: fake_map
    libnrt.nc_to_real_nc = lambda dev, i: fake_map[(dev, i)]
    bass_interp.nc_to_real_nc = libnrt.nc_to_real_nc
    bass_interp.pnc_id_to_device_and_real_nc_index = (
        lambda core_id: (core_id // 8, fake_map[(core_id // 8, core_id % 8)])
    )
    fake_rid = {d: d for d in range(16)}
    libnrt.get_device_id_to_routing_id_mapping = lambda: fake_rid
    bass_interp.get_device_id_to_routing_id_mapping = lambda: fake_rid

    rng = np.random.default_rng(0)
    source = rng.integers(0, VOCAB, (B, nsteps)).astype(np.int32)
    emb = rng.standard_normal((VOCAB, EMB), np.float32)
    W_ih = (rng.standard_normal((4 * HID, EMB), np.float32) / np.sqrt(EMB)).astype(np.float32)
    W_hh = (rng.standard_normal((4 * HID, HID), np.float32) / np.sqrt(HID)).astype(np.float32)
    b_ih = np.zeros(4 * HID, np.float32)
    b_hh = np.zeros(4 * HID, np.float32)

    nc = build(nsteps, exchange)
    in_maps = prepare_in_maps(source, emb, W_ih, W_hh, b_ih, b_hh, nsteps)

    sim = bass_interp.MultiCoreSim(nc, CORES)
    for i in range(CORES):
        for k, v in in_maps[i].items():
            sim.cores[i].tensor(k)[:] = v
    sim.simulate(check_with_hw=check_with_hw)

    outs = [
        np.array(sim.cores[i].mem_tensor("out")).reshape(2, 128, B)
        for i in range(CORES)
    ]
    h = np.concatenate([o[0].T for o in outs], axis=1)
    c = np.concatenate([o[1].T for o in outs], axis=1)
    actual = np.stack([h, c])

    # numpy reference
    X = emb[source]  # [B, S, E]
    hh = np.zeros((B, HID), np.float32)
    cc = np.zeros((B, HID), np.float32)
    for t in range(nsteps):
        gates = X[:, t, :] @ W_ih.T + hh @ W_hh.T + b_ih + b_hh
        i_, f_, g_, o_ = np.split(gates, 4, axis=-1)
        i_ = 1 / (1 + np.exp(-i_))
        f_ = 1 / (1 + np.exp(-f_))
        g_ = np.tanh(g_)
        o_ = 1 / (1 + np.exp(-o_))
        cc = f_ * cc + i_ * g_
        hh = o_ * np.tanh(cc)
    expected = np.stack([hh, cc])
    err = np.abs(actual - expected).max() / np.abs(expected).max()
    times = [sim.cores[i].time for i in range(CORES)]
    print(f"sim nsteps={nsteps} absmax_rel_err={err:.3e} sim_time_ns={max(times)}")
    return err


if __name__ == "__main__":
    ns = int(sys.argv[1]) if len(sys.argv) > 1 else 8
    ex = sys.argv[2] if len(sys.argv) > 2 else "remote"
    _simulate(ns, ex)



# revision 2
# speedup vs baseline: 1.0295x; 1.0295x over previous
"""LSTM encoder (B=64, S=512, E=H=1024) on 8 trn2 NeuronCores — v2.

Tensor-parallel over the 4H gate dim (128 h-channels x 4 gates = 512 gate
rows per core), with three structural changes vs the v1 baseline:

1. PSUM-direct phase 1: the input projection gx = W_ih X + b accumulates
   directly into the psum bank that the recurrent matmuls later add onto.
   No identity matmuls, no gx DRAM roundtrip, no psum->stage copies.
   Psum ring: 2 tile-parities x 4 gate banks; tile tau (8 steps) occupies
   parity tau%2. Phase-1 MMs for tile tau+1 are interleaved into tile
   tau's steps (4-5 per step, after the rec MMs).
2. Single-broadcast exchange: one remote_dma_broadcast per step with all
   8 relative dests (self included) whose out_ap slot is register-offset
   (DynSlice) by the sender's physical id. Receiver slots are thus
   sender-physical-keyed and the W_hh column layout is uniform across
   cores (slot d holds logical chunk _M[d]). 1 Q7 prep (~1us) per step
   instead of 7 (~7us).
3. Latency-ordered gates: chunk order [i, g, f, o]; per-gate activations
   (sigmoid i early, o last) so the c-update chain overlaps the matmul
   stream; bias is folded into phase 1 via a 9th constant-one K chunk.

Self-contained: hardcodes all shapes; host-side prep is numpy only.
"""

import sys

sys.path.insert(0, "/opt/trn_rl_repo")

import numpy as np
import ml_dtypes

import concourse.bass as bass
import concourse.bacc as bacc
import concourse.mybir as mybir
from concourse.bass_types import DynSlice
from bass_rust import RuntimeValue

BF16 = ml_dtypes.bfloat16
F8 = ml_dtypes.float8_e4m3
AF = mybir.ActivationFunctionType
dt = mybir.dt

VOCAB, EMB, HID = 32000, 1024, 1024
B = 64
S = 512
CORES = 8
KC = 8             # contraction chunks of 128
NCHUNK = 4         # gate chunks per core; chunk order: i, g, f, o
G = NCHUNK * 128   # 512 gate rows per core
NT = 512           # tokens per phase-1 tile (8 steps x 64 batch)
TPT = NT // B      # 8 steps per tile
NIDX = NT // 16    # idx columns per tile
# pytorch gate blocks in W rows: i, f, g, o ; our chunk order: i, g, f, o
CHUNK_TO_BLOCK = [0, 2, 1, 3]
# logical replica -> physical TPB on trn2 (driver V0 table); involution.
_M = [0, 1, 2, 3, 6, 7, 4, 5]


def build(nsteps=S, whh_fp8=False):
    TT = B * nsteps // NT
    assert B * nsteps % NT == 0 and TT >= 2
    nc = bacc.Bacc(None, target_bir_lowering=False, num_swdge_queues=2)

    whh_dt = dt.float8e4 if whh_fp8 else dt.bfloat16

    emb_d = nc.declare_dram_parameter("emb16", [VOCAB, EMB], dt.bfloat16, isOutput=False)
    idx_d = nc.declare_dram_parameter("idx", [TT, 128, NIDX], dt.int16, isOutput=False)
    wih_d = nc.declare_dram_parameter("w_ih", [128, KC * G], dt.bfloat16, isOutput=False)
    wb_d = nc.declare_dram_parameter("w_b", [128, G], dt.bfloat16, isOutput=False)
    whh_d = nc.declare_dram_parameter("w_hh", [128, KC * G], whh_dt, isOutput=False)
    slot_d = nc.declare_dram_parameter("slotoff", [1, 1], dt.int32, isOutput=False)
    out_d = nc.declare_dram_parameter("out", [2, 128, B], dt.float32, isOutput=True)

    bar_in = nc.dram_tensor("bar_in", [128, 4], dt.float32)
    bar_out = nc.dram_tensor("bar_out", [128, 4], dt.float32, addr_space="Shared")

    cc_sem = nc.alloc_semaphore("cc_sem")
    bar_sem = nc.alloc_semaphore("bar_sem")
    bardma_sem = nc.alloc_semaphore("bardma_sem")
    wload = nc.alloc_semaphore("wload")
    g_sem = [nc.alloc_semaphore(f"g_sem{i}") for i in range(3)]
    mm1 = nc.alloc_semaphore("mm1")    # +1 per finished ph1 tile
    mmr = nc.alloc_semaphore("mmr")    # +4 per step (rec MM per gate chunk)
    act_s = nc.alloc_semaphore("act_s")  # +5 per step
    dve_s = nc.alloc_semaphore("dve_s")  # +1 init, +4 per step
    prep_s = nc.alloc_semaphore("prep_s")
    rsem = [nc.alloc_semaphore("rsem0"), nc.alloc_semaphore("rsem1")]
    lsem = [nc.alloc_semaphore("lsem0"), nc.alloc_semaphore("lsem1")]
    fin = nc.alloc_semaphore("fin")
    init_s = nc.alloc_semaphore("init_s")

    from contextlib import ExitStack

    with ExitStack() as ctx:
        sb = lambda name, shape, d: ctx.enter_context(nc.sbuf_tensor(name, shape, d))
        idx_sb = sb("idx_sb", [128, TT * NIDX], dt.int16)
        wih_sb = sb("wih_sb", [128, KC * G], dt.bfloat16)
        wb_sb = sb("wb_sb", [128, G], dt.bfloat16)
        whh_sb = sb("whh_sb", [128, KC * G], whh_dt)
        slot_sb = sb("slot_sb", [1, 1], dt.int32)
        xt = [sb(f"xt{i}", [128, KC, NT], dt.bfloat16) for i in range(3)]
        xt9 = sb("xt9", [128, NT], dt.bfloat16)   # constant-1 row 0 (bias chunk)
        hg = [sb(f"hg{i}", [128, CORES * B], dt.bfloat16) for i in range(2)]
        hsrc = [sb(f"hsrc{i}", [128, B], dt.bfloat16) for i in range(2)]
        sg = sb("sg", [128, NCHUNK * B], dt.float32)
        ig_sb = sb("ig_sb", [128, B], dt.float32)
        fc_sb = sb("fc_sb", [128, B], dt.float32)
        thc_sb = sb("thc_sb", [128, B], dt.float32)
        c_sb = sb("c_sb", [128, B], dt.float32)
        hout_sb = sb("hout_sb", [128, B], dt.float32)
        bar_sb = sb("bar_sb", [128, 4], dt.float32)
        # 2 psum tensors of 4 banks each: parity x [gate chunk, step%8, batch]
        ps = [
            ctx.enter_context(nc.psum_tensor(f"ps{i}", [128, NCHUNK * 512], dt.float32))
            for i in range(2)
        ]
        block = ctx.enter_context(nc.Block())

        def ph1_mm(te, tau, cb, k, inc_mm1):
            """One phase-1 MM: psum[tau%2] bank cb += W chunk k x xt."""
            lhsT = (
                wih_sb[:, G * k + 128 * cb : G * k + 128 * (cb + 1)]
                if k < KC
                else wb_sb[:, 128 * cb : 128 * (cb + 1)]
            )
            rhs = xt[tau % 3][:, k, :] if k < KC else xt9[:, :]
            mm = te.matmul(
                ps[tau % 2][:, 512 * cb : 512 * (cb + 1)],
                lhsT=lhsT,
                rhs=rhs,
                start=(k == 0),
                stop=False,
                skip_group_check=True,
            )
            if inc_mm1:
                mm.then_inc(mm1, 1)

        # =========== SYNC: loads + final store ===========
        @block.sync
        def _(sy):
            sy.dma_start(
                out=idx_sb.ap().rearrange("p (t c) -> p t c", t=TT),
                in_=idx_d.ap().rearrange("t p c -> p t c"),
            ).then_inc(wload, 16)
            sy.dma_start(out=wih_sb[:, :], in_=wih_d[:, :]).then_inc(wload, 16)
            sy.dma_start(out=wb_sb[:, :], in_=wb_d[:, :]).then_inc(wload, 16)
            sy.dma_start(out=whh_sb[:, :], in_=whh_d[:, :]).then_inc(wload, 16)
            sy.dma_start(out=slot_sb[:, :], in_=slot_d[:, :]).then_inc(wload, 16)
            sy.wait_ge(dve_s, 1 + 4 * nsteps)
            sy.dma_start(out=out_d[0, :, :], in_=hout_sb[:, :]).then_inc(fin, 16)
            sy.dma_start(out=out_d[1, :, :], in_=c_sb[:, :]).then_inc(fin, 16)
            sy.wait_ge(fin, 32)

        # =========== GPSIMD: barrier, gathers, exchange ===========
        @block.gpsimd
        def _(gp):
            gp.memset(bar_sb[:, :], 0.0).then_inc(bar_sem, 1)
            gp.wait_ge(bar_sem, 1)
            gp.dma_start(out=bar_in[:, :], in_=bar_sb[:, :]).then_inc(bardma_sem, 16)
            gp.wait_ge(bardma_sem, 16)
            gp.collective_compute(
                "AllReduce",
                mybir.AluOpType.add,
                ins=[bar_in.ap().opt()],
                outs=[bar_out.ap().opt()],
                replica_groups=[list(range(CORES))],
            ).then_inc(cc_sem, 1)
            gp.wait_ge(cc_sem, 1)
            gp.wait_ge(wload, 80)

            slot_reg = gp.alloc_register("slot_reg")
            gp.reg_load(slot_reg, slot_sb[0:1, 0:1])

            def bcast_preps(exchanges):
                """Preps for several exchanges in ONE switch (8 static arms).

                Exchange e delivers h(e) into hg[(e+1)%2], read from
                hsrc[e%2]. Ring order inside the arm = ascending e, matching
                the one-trigger-per-step FIFO consumption."""
                for s in gp.Switch(RuntimeValue(slot_reg), CORES):
                    for e in exchanges:
                        gp.remote_dma_broadcast(
                            out_ap=hg[(e + 1) % 2][:, B * s : B * (s + 1)],
                            in_ap=hsrc[e % 2][:, :],
                            remote_sem=rsem[(e + 1) % 2],
                            local_sem=lsem[e % 2],
                            rdests=[(0, d) for d in range(CORES)],
                        ).then_inc(prep_s, 1)

            # prologue gathers: tiles 0..2 (immediate mode, queue 1)
            for tau in range(min(3, TT)):
                gp.dma_gather(
                    out_ap=xt[tau % 3][:, :, :],
                    in_ap=emb_d[:, :],
                    idxs_ap=idx_sb[:, NIDX * tau : NIDX * (tau + 1)],
                    num_idxs=NT,
                    num_idxs_reg=NT,
                    elem_size=EMB,
                    transpose=True,
                    queue_num=1,
                ).then_inc(g_sem[tau % 3], 16)

            # exchanges exist for e in 0..nsteps-2; prep in batches of 4
            PB = 4
            last_e = nsteps - 2
            bcast_preps(range(0, min(PB, last_e + 1)))

            for t in range(nsteps - 1):
                if t % PB == 0 and t + PB <= last_e:
                    bcast_preps(range(t + PB, min(t + 2 * PB, last_e + 1)))
                gp.wait_ge(prep_s, t + 1)
                gp.wait_ge(dve_s, 1 + 4 * t + 4)  # h(t) in hsrc[t%2]
                gp.trigger_dma(count=1)
                # steady-state gathers: tile t//8 + 3 launched at step 8k+2
                if t % TPT == 2 and t // TPT + 3 < TT:
                    tau = t // TPT + 3
                    gp.wait_ge(mm1, tau - 2)  # xt[tau%3] free (tile tau-3 done)
                    gp.dma_gather(
                        out_ap=xt[tau % 3][:, :, :],
                        in_ap=emb_d[:, :],
                        idxs_ap=idx_sb[:, NIDX * tau : NIDX * (tau + 1)],
                        num_idxs=NT,
                        num_idxs_reg=NT,
                        elem_size=EMB,
                        transpose=True,
                        queue_num=1,
                    ).then_inc(g_sem[tau % 3], 16)

        # =========== TENSOR ===========
        @block.tensor
        def _(te):
            te.wait_ge(wload, 80)
            te.wait_ge(dve_s, 1)  # xt9 + c memsets done
            # prologue: full ph1 for tiles 0 and 1
            for tau in range(2):
                te.wait_ge(g_sem[tau % 3], 16)
                for cb in range(NCHUNK):
                    for k in range(KC + 1):
                        ph1_mm(te, tau, cb, k, inc_mm1=(cb == NCHUNK - 1 and k == KC))

            for t in range(nsteps):
                P = (t // TPT) % 2
                c0 = 64 * (t % TPT)
                hpar = t % 2
                if t >= 1:
                    te.wait_ge(rsem[hpar], 16 * ((t + 1) // 2))
                    for cb in range(NCHUNK):
                        for d in range(CORES):
                            mm = te.matmul(
                                ps[P][:, 512 * cb + c0 : 512 * cb + c0 + 64],
                                lhsT=whh_sb[:, G * d + 128 * cb : G * d + 128 * (cb + 1)],
                                rhs=hg[hpar][:, B * d : B * (d + 1)],
                                start=False,
                                stop=(d == CORES - 1),
                                skip_group_check=True,
                            )
                        mm.then_inc(mmr, 1)
                else:
                    # t=0: gates = gx only; bump mmr so ACT waits stay uniform
                    te.nop().then_inc(mmr, 4)
                # interleaved ph1 for tile t//8 + 1 (2..TT-1)
                tf = t // TPT + 1
                if 2 <= tf < TT:
                    s = t % TPT
                    lo, hi = (36 * s) // TPT, (36 * (s + 1)) // TPT
                    for idx in range(lo, hi):
                        cb, k = idx // (KC + 1), idx % (KC + 1)
                        if idx == 0:
                            te.wait_ge(g_sem[tf % 3], 16 * (tf // 3 + 1))
                            # psum parity reuse: tile tf-2 fully consumed
                            # (never blocks in practice; needed for the
                            # happens-before graph)
                            te.wait_ge(mmr, 32 * tf - 32)
                            te.wait_ge(act_s, 40 * tf - 41)
                        ph1_mm(te, tf, cb, k, inc_mm1=(idx == 35))

        # =========== SCALAR (ACT) ===========
        # per step: sig(i), tanh(g), sig(f), sig(o), tanh(c)  [+5]
        @block.scalar
        def _(sc):
            sc.wait_ge(wload, 80)
            for t in range(nsteps):
                P = (t // TPT) % 2
                c0 = 64 * (t % TPT)
                if t == 0:
                    sc.wait_ge(mm1, 1)
                base = 4 * t  # rec(t) gate cb done at mmr = 4t + cb + 1

                def gate_in(cb):
                    return ps[P][:, 512 * cb + c0 : 512 * cb + c0 + 64]

                if t >= 1:
                    sc.wait_ge(mmr, base + 1)
                sc.activation(sg[:, 0:B], gate_in(0), AF.Sigmoid).then_inc(act_s, 1)
                if t >= 1:
                    sc.wait_ge(mmr, base + 2)
                sc.activation(sg[:, B : 2 * B], gate_in(1), AF.Tanh).then_inc(act_s, 1)
                if t >= 1:
                    sc.wait_ge(mmr, base + 3)
                sc.activation(sg[:, 2 * B : 3 * B], gate_in(2), AF.Sigmoid).then_inc(act_s, 1)
                if t >= 1:
                    sc.wait_ge(mmr, base + 4)
                sc.activation(sg[:, 3 * B : 4 * B], gate_in(3), AF.Sigmoid).then_inc(act_s, 1)
                sc.wait_ge(dve_s, 1 + 4 * t + 3)  # c(t) updated
                sc.activation(thc_sb[:, :], c_sb[:, :], AF.Tanh).then_inc(act_s, 1)

        # =========== VECTOR (DVE) ===========
        @block.vector
        def _(ve):
            ve.memset(xt9[:, :], 0.0).then_inc(init_s, 1)
            ve.wait_ge(init_s, 1)
            ve.memset(xt9[0:1, :], 1.0)
            ve.memset(c_sb[:, :], 0.0).then_inc(dve_s, 1)
            for t in range(nsteps):
                ve.wait_ge(act_s, 5 * t + 2)  # sig(i), tanh(g)
                if t >= 1:
                    ve.wait_ge(dve_s, 4 * t)  # ig_sb/fc_sb reads by add(t-1)
                ve.tensor_mul(ig_sb[:, :], sg[:, 0:B], sg[:, B : 2 * B]).then_inc(dve_s, 1)
                ve.wait_ge(act_s, 5 * t + 3)  # sig(f)
                ve.wait_ge(dve_s, 4 * t if t >= 1 else 1)  # c_sb from add(t-1)/memset
                ve.tensor_mul(fc_sb[:, :], sg[:, 2 * B : 3 * B], c_sb[:, :]).then_inc(dve_s, 1)
                ve.wait_ge(dve_s, 4 * t + 3)  # ig, fc writebacks landed
                if t >= 1:
                    ve.wait_ge(act_s, 5 * t)  # tanh(c) of t-1 read c_sb
                ve.tensor_add(c_sb[:, :], ig_sb[:, :], fc_sb[:, :]).then_inc(dve_s, 1)
                ve.wait_ge(act_s, 5 * t + 5)  # sig(o), tanh(c)
                if t == nsteps - 1:
                    ve.tensor_mul(hout_sb[:, :], sg[:, 3 * B : 4 * B], thc_sb[:, :]).then_inc(dve_s, 1)
                else:
                    if t >= 2:
                        ve.wait_ge(lsem[t % 2], 16 * (t // 2))  # hsrc[t%2] drained
                    ve.tensor_mul(hsrc[t % 2][:, :], sg[:, 3 * B : 4 * B], thc_sb[:, :]).then_inc(dve_s, 1)

    nc.compile()
    return nc


# ---------------------------------------------------------------------------
# host-side input prep
# ---------------------------------------------------------------------------

def prepare_in_maps(source, emb, W_ih, W_hh, b_ih, b_hh, nsteps=S, whh_fp8=False):
    source = np.asarray(source)
    emb = np.asarray(emb, np.float32)
    W_ih = np.asarray(W_ih, np.float32)
    W_hh = np.asarray(W_hh, np.float32)
    b = np.asarray(b_ih, np.float32) + np.asarray(b_hh, np.float32)

    TT = B * nsteps // NT
    emb16 = emb.astype(BF16)

    idx = np.zeros([TT, 128, NIDX], np.int16)
    j = np.arange(NT)
    tprime, bb = j // B, j % B
    for tau in range(TT):
        ids = source[bb, TPT * tau + tprime].astype(np.int16)
        wrapped = ids.reshape(NIDX, 16).T  # [16, NIDX]
        idx[tau] = np.tile(wrapped, (8, 1))

    H = HID
    in_maps = []
    for jc in range(CORES):
        rows = np.concatenate(
            [
                np.arange(CHUNK_TO_BLOCK[cb] * H + 128 * jc,
                          CHUNK_TO_BLOCK[cb] * H + 128 * (jc + 1))
                for cb in range(NCHUNK)
            ]
        )
        Wi = W_ih[rows]           # [512, 1024]
        Wh = W_hh[rows]
        bi = b[rows]              # [512]

        wi4 = Wi.reshape(NCHUNK, 128, KC, 128)           # [cb, m, k, p]
        wih = np.transpose(wi4, (3, 2, 0, 1)).reshape(128, KC * G).astype(BF16)

        wb = np.zeros([128, G], np.float32)
        wb[0] = bi
        wb = wb.astype(BF16)

        # whh: slot d holds logical chunk _M[d]; uniform across cores
        wh4 = Wh.reshape(NCHUNK, 128, KC, 128)           # [cb, m, k, p]
        wh4p = wh4[:, :, _M, :]
        whh = np.transpose(wh4p, (3, 2, 0, 1)).reshape(128, KC * G)
        whh = whh.astype(F8) if whh_fp8 else whh.astype(BF16)

        in_maps.append(
            {
                "emb16": emb16,
                "idx": idx,
                "w_ih": wih,
                "w_b": wb,
                "w_hh": whh,
                "slotoff": np.array([[_M[jc]]], np.int32),
            }
        )
    return in_maps


import os

_BUILD_CACHE = {}
WHH_FP8 = os.environ.get("WHH_FP8", "0") == "1"


def _get_nc(nsteps=S, whh_fp8=WHH_FP8):
    key = (nsteps, whh_fp8)
    if key not in _BUILD_CACHE:
        _BUILD_CACHE[key] = build(nsteps, whh_fp8)
    return _BUILD_CACHE[key]


def kernel(source, emb, W_ih, W_hh, b_ih, b_hh, _trace=False):
    from concourse.bass_utils import run_bass_kernel_spmd

    nc = _get_nc()
    in_maps = prepare_in_maps(source, emb, W_ih, W_hh, b_ih, b_hh, whh_fp8=WHH_FP8)
    res = run_bass_kernel_spmd(nc, in_maps, core_ids=list(range(CORES)), trace=_trace)
    outs = [res.results[i]["out"] for i in range(CORES)]  # each [2, 128, B]
    h = np.concatenate([o[0].T for o in outs], axis=1)  # [B, 8*128]
    c = np.concatenate([o[1].T for o in outs], axis=1)
    out = np.stack([h, c]).astype(np.float32)
    if _trace:
        return out, res
    return out


# ---------------------------------------------------------------------------
# dev: multi-core simulation on a reduced problem
# ---------------------------------------------------------------------------

def _simulate(nsteps=16, whh_fp8=False):
    from concourse import bass_interp, libnrt

    libnrt.get_trn2_nc_mapping.cache_clear()
    libnrt.nc_to_real_nc.cache_clear()
    fake_map = {(d, i): _M[i] for d in range(16) for i in range(8)}
    libnrt.get_trn2_nc_mapping = lambda: fake_map
    libnrt.nc_to_real_nc = lambda dev, i: fake_map[(dev, i)]
    bass_interp.nc_to_real_nc = libnrt.nc_to_real_nc
    bass_interp.pnc_id_to_device_and_real_nc_index = (
        lambda core_id: (core_id // 8, fake_map[(core_id // 8, core_id % 8)])
    )
    fake_rid = {d: d for d in range(16)}
    libnrt.get_device_id_to_routing_id_mapping = lambda: fake_rid
    bass_interp.get_device_id_to_routing_id_mapping = lambda: fake_rid

    rng = np.random.default_rng(0)
    source = rng.integers(0, VOCAB, (B, nsteps)).astype(np.int32)
    emb = rng.standard_normal((VOCAB, EMB), np.float32)
    W_ih = (rng.standard_normal((4 * HID, EMB), np.float32) / np.sqrt(EMB)).astype(np.float32)
    W_hh = (rng.standard_normal((4 * HID, HID), np.float32) / np.sqrt(HID)).astype(np.float32)
    b_ih = np.zeros(4 * HID, np.float32)
    b_hh = np.zeros(4 * HID, np.float32)

    nc = build(nsteps, whh_fp8)
    in_maps = prepare_in_maps(source, emb, W_ih, W_hh, b_ih, b_hh, nsteps, whh_fp8)

    sim = bass_interp.MultiCoreSim(nc, CORES)
    for i in range(CORES):
        for k, v in in_maps[i].items():
            sim.cores[i].tensor(k)[:] = v
    sim.simulate()

    outs = [
        np.array(sim.cores[i].mem_tensor("out")).reshape(2, 128, B)
        for i in range(CORES)
    ]
    h = np.concatenate([o[0].T for o in outs], axis=1)
    c = np.concatenate([o[1].T for o in outs], axis=1)
    actual = np.stack([h, c])

    X = emb[source]
    hh = np.zeros((B, HID), np.float32)
    cc = np.zeros((B, HID), np.float32)
    for t in range(nsteps):
        gates = X[:, t, :] @ W_ih.T + hh @ W_hh.T + b_ih + b_hh
        i_, f_, g_, o_ = np.split(gates, 4, axis=-1)
        i_ = 1 / (1 + np.exp(-i_))
        f_ = 1 / (1 + np.exp(-f_))
        g_ = np.tanh(g_)
        o_ = 1 / (1 + np.exp(-o_))
        cc = f_ * cc + i_ * g_
        hh = o_ * np.tanh(cc)
    expected = np.stack([hh, cc])
    err = np.abs(actual - expected).max() / np.abs(expected).max()
    times = [sim.cores[i].time for i in range(CORES)]
    print(f"sim nsteps={nsteps} fp8={whh_fp8} absmax_rel_err={err:.3e} sim_time_ns={max(times)}")
    return err


if __name__ == "__main__":
    ns = int(sys.argv[1]) if len(sys.argv) > 1 else 16
    fp8 = len(sys.argv) > 2 and sys.argv[2] == "fp8"
    _simulate(ns, fp8)


# revision 3
# speedup vs baseline: 1.0317x; 1.0021x over previous
"""LSTM encoder (B=64, S=512, E=H=1024) on 8 trn2 NeuronCores — v2.

Tensor-parallel over the 4H gate dim (128 h-channels x 4 gates = 512 gate
rows per core), with three structural changes vs the v1 baseline:

1. PSUM-direct phase 1: the input projection gx = W_ih X + b accumulates
   directly into the psum bank that the recurrent matmuls later add onto.
   No identity matmuls, no gx DRAM roundtrip, no psum->stage copies.
   Psum ring: 2 tile-parities x 4 gate banks; tile tau (8 steps) occupies
   parity tau%2. Phase-1 MMs for tile tau+1 are interleaved into tile
   tau's steps (4-5 per step, after the rec MMs).
2. Single-broadcast exchange: one remote_dma_broadcast per step with all
   8 relative dests (self included) whose out_ap slot is register-offset
   (DynSlice) by the sender's physical id. Receiver slots are thus
   sender-physical-keyed and the W_hh column layout is uniform across
   cores (slot d holds logical chunk _M[d]). 1 Q7 prep (~1us) per step
   instead of 7 (~7us).
3. Latency-ordered gates: chunk order [i, g, f, o]; per-gate activations
   (sigmoid i early, o last) so the c-update chain overlaps the matmul
   stream; bias is folded into phase 1 via a 9th constant-one K chunk.

Self-contained: hardcodes all shapes; host-side prep is numpy only.
"""

import sys

sys.path.insert(0, "/opt/trn_rl_repo")

import numpy as np
import ml_dtypes

import concourse.bass as bass
import concourse.bacc as bacc
import concourse.mybir as mybir
from concourse.bass_types import DynSlice
from bass_rust import RuntimeValue

BF16 = ml_dtypes.bfloat16
F8 = ml_dtypes.float8_e4m3
AF = mybir.ActivationFunctionType
dt = mybir.dt

VOCAB, EMB, HID = 32000, 1024, 1024
B = 64
S = 512
CORES = 8
KC = 8             # contraction chunks of 128
NCHUNK = 4         # gate chunks per core; chunk order: i, f, g, o
G = NCHUNK * 128   # 512 gate rows per core
NT = 512           # tokens per phase-1 tile (8 steps x 64 batch)
TPT = NT // B      # 8 steps per tile
NIDX = NT // 16    # idx columns per tile
# pytorch gate blocks in W rows: i, f, g, o ; our chunk order: i, f, g, o
CHUNK_TO_BLOCK = [0, 1, 2, 3]
# logical replica -> physical TPB on trn2 (driver V0 table); involution.
_M = [0, 1, 2, 3, 6, 7, 4, 5]


def build(nsteps=S, whh_fp8=False):
    TT = B * nsteps // NT
    assert B * nsteps % NT == 0 and TT >= 2
    nc = bacc.Bacc(None, target_bir_lowering=False, num_swdge_queues=2)

    whh_dt = dt.float8e4 if whh_fp8 else dt.bfloat16

    emb_d = nc.declare_dram_parameter("emb16", [VOCAB, EMB], dt.bfloat16, isOutput=False)
    idx_d = nc.declare_dram_parameter("idx", [TT, 128, NIDX], dt.int16, isOutput=False)
    wih_d = nc.declare_dram_parameter("w_ih", [128, KC * G], dt.bfloat16, isOutput=False)
    wb_d = nc.declare_dram_parameter("w_b", [128, G], dt.bfloat16, isOutput=False)
    whh_d = nc.declare_dram_parameter("w_hh", [128, KC * G], whh_dt, isOutput=False)
    slot_d = nc.declare_dram_parameter("slotoff", [1, 1], dt.int32, isOutput=False)
    out_d = nc.declare_dram_parameter("out", [2, 128, B], dt.float32, isOutput=True)

    bar_in = nc.dram_tensor("bar_in", [128, 4], dt.float32)
    bar_out = nc.dram_tensor("bar_out", [128, 4], dt.float32, addr_space="Shared")

    cc_sem = nc.alloc_semaphore("cc_sem")
    bar_sem = nc.alloc_semaphore("bar_sem")
    bardma_sem = nc.alloc_semaphore("bardma_sem")
    wload = nc.alloc_semaphore("wload")
    g_sem = [nc.alloc_semaphore(f"g_sem{i}") for i in range(3)]
    mm1 = nc.alloc_semaphore("mm1")    # +1 per finished ph1 tile
    mmr = nc.alloc_semaphore("mmr")    # +4 per step (rec MM per gate chunk)
    act_s = nc.alloc_semaphore("act_s")  # +4 per step
    dve_s = nc.alloc_semaphore("dve_s")  # +1 init, +4 per step
    prep_s = nc.alloc_semaphore("prep_s")
    rsem = [nc.alloc_semaphore("rsem0"), nc.alloc_semaphore("rsem1")]
    lsem = [nc.alloc_semaphore("lsem0"), nc.alloc_semaphore("lsem1")]
    fin = nc.alloc_semaphore("fin")
    init_s = nc.alloc_semaphore("init_s")

    from contextlib import ExitStack

    with ExitStack() as ctx:
        sb = lambda name, shape, d: ctx.enter_context(nc.sbuf_tensor(name, shape, d))
        idx_sb = sb("idx_sb", [128, TT * NIDX], dt.int16)
        wih_sb = sb("wih_sb", [128, KC * G], dt.bfloat16)
        wb_sb = sb("wb_sb", [128, G], dt.bfloat16)
        whh_sb = sb("whh_sb", [128, KC * G], whh_dt)
        slot_sb = sb("slot_sb", [1, 1], dt.int32)
        xt = [sb(f"xt{i}", [128, KC, NT], dt.bfloat16) for i in range(3)]
        xt9 = sb("xt9", [128, NT], dt.bfloat16)   # constant-1 row 0 (bias chunk)
        hg = [sb(f"hg{i}", [128, CORES * B], dt.bfloat16) for i in range(2)]
        hsrc = [sb(f"hsrc{i}", [128, B], dt.bfloat16) for i in range(2)]
        sg = sb("sg", [128, NCHUNK * B], dt.float32)
        ig_sb = sb("ig_sb", [128, B], dt.float32)
        fc_sb = sb("fc_sb", [128, B], dt.float32)
        thc_sb = sb("thc_sb", [128, B], dt.float32)
        c_sb = sb("c_sb", [128, B], dt.float32)
        hout_sb = sb("hout_sb", [128, B], dt.float32)
        bar_sb = sb("bar_sb", [128, 4], dt.float32)
        # 2 psum tensors of 4 banks each: parity x [gate chunk, step%8, batch]
        ps = [
            ctx.enter_context(nc.psum_tensor(f"ps{i}", [128, NCHUNK * 512], dt.float32))
            for i in range(2)
        ]
        block = ctx.enter_context(nc.Block())

        def ph1_mm(te, tau, cb, k, inc_mm1):
            """One phase-1 MM: psum[tau%2] bank cb += W chunk k x xt."""
            lhsT = (
                wih_sb[:, G * k + 128 * cb : G * k + 128 * (cb + 1)]
                if k < KC
                else wb_sb[:, 128 * cb : 128 * (cb + 1)]
            )
            rhs = xt[tau % 3][:, k, :] if k < KC else xt9[:, :]
            mm = te.matmul(
                ps[tau % 2][:, 512 * cb : 512 * (cb + 1)],
                lhsT=lhsT,
                rhs=rhs,
                start=(k == 0),
                stop=False,
                skip_group_check=True,
            )
            if inc_mm1:
                mm.then_inc(mm1, 1)

        # =========== SYNC: loads + final store ===========
        @block.sync
        def _(sy):
            sy.dma_start(
                out=idx_sb.ap().rearrange("p (t c) -> p t c", t=TT),
                in_=idx_d.ap().rearrange("t p c -> p t c"),
            ).then_inc(wload, 16)
            sy.dma_start(out=wih_sb[:, :], in_=wih_d[:, :]).then_inc(wload, 16)
            sy.dma_start(out=wb_sb[:, :], in_=wb_d[:, :]).then_inc(wload, 16)
            sy.dma_start(out=whh_sb[:, :], in_=whh_d[:, :]).then_inc(wload, 16)
            sy.dma_start(out=slot_sb[:, :], in_=slot_d[:, :]).then_inc(wload, 16)
            sy.wait_ge(dve_s, 1 + 4 * nsteps)
            sy.dma_start(out=out_d[0, :, :], in_=hout_sb[:, :]).then_inc(fin, 16)
            sy.dma_start(out=out_d[1, :, :], in_=c_sb[:, :]).then_inc(fin, 16)
            sy.wait_ge(fin, 32)

        # =========== GPSIMD: barrier, gathers, exchange ===========
        @block.gpsimd
        def _(gp):
            gp.memset(bar_sb[:, :], 0.0).then_inc(bar_sem, 1)
            gp.wait_ge(bar_sem, 1)
            gp.dma_start(out=bar_in[:, :], in_=bar_sb[:, :]).then_inc(bardma_sem, 16)
            gp.wait_ge(bardma_sem, 16)
            gp.collective_compute(
                "AllReduce",
                mybir.AluOpType.add,
                ins=[bar_in.ap().opt()],
                outs=[bar_out.ap().opt()],
                replica_groups=[list(range(CORES))],
            ).then_inc(cc_sem, 1)
            gp.wait_ge(cc_sem, 1)
            gp.wait_ge(wload, 80)

            slot_reg = gp.alloc_register("slot_reg")
            gp.reg_load(slot_reg, slot_sb[0:1, 0:1])

            def bcast_preps(exchanges):
                """Preps for several exchanges in ONE switch (8 static arms).

                Exchange e delivers h(e) into hg[(e+1)%2], read from
                hsrc[e%2]. Ring order inside the arm = ascending e, matching
                the one-trigger-per-step FIFO consumption."""
                for s in gp.Switch(RuntimeValue(slot_reg), CORES):
                    for e in exchanges:
                        gp.remote_dma_broadcast(
                            out_ap=hg[(e + 1) % 2][:, B * s : B * (s + 1)],
                            in_ap=hsrc[e % 2][:, :],
                            remote_sem=rsem[(e + 1) % 2],
                            local_sem=lsem[e % 2],
                            rdests=[(0, d) for d in range(CORES)],
                        ).then_inc(prep_s, 1)

            # prologue gathers: tiles 0..2 (immediate mode, queue 1)
            for tau in range(min(3, TT)):
                gp.dma_gather(
                    out_ap=xt[tau % 3][:, :, :],
                    in_ap=emb_d[:, :],
                    idxs_ap=idx_sb[:, NIDX * tau : NIDX * (tau + 1)],
                    num_idxs=NT,
                    num_idxs_reg=NT,
                    elem_size=EMB,
                    transpose=True,
                    queue_num=1,
                ).then_inc(g_sem[tau % 3], 16)

            # exchanges exist for e in 0..nsteps-2; prep in batches of 4
            PB = 4
            last_e = nsteps - 2
            bcast_preps(range(0, min(PB, last_e + 1)))

            for t in range(nsteps - 1):
                if t % PB == 0 and t + PB <= last_e:
                    bcast_preps(range(t + PB, min(t + 2 * PB, last_e + 1)))
                gp.wait_ge(prep_s, t + 1)
                gp.wait_ge(dve_s, 1 + 4 * t + 4)  # h(t) in hsrc[t%2]
                gp.trigger_dma(count=1)
                # steady-state gathers: tile t//8 + 3 launched at step 8k+2
                if t % TPT == 2 and t // TPT + 3 < TT:
                    tau = t // TPT + 3
                    gp.wait_ge(mm1, tau - 2)  # xt[tau%3] free (tile tau-3 done)
                    gp.dma_gather(
                        out_ap=xt[tau % 3][:, :, :],
                        in_ap=emb_d[:, :],
                        idxs_ap=idx_sb[:, NIDX * tau : NIDX * (tau + 1)],
                        num_idxs=NT,
                        num_idxs_reg=NT,
                        elem_size=EMB,
                        transpose=True,
                        queue_num=1,
                    ).then_inc(g_sem[tau % 3], 16)

        # =========== TENSOR ===========
        @block.tensor
        def _(te):
            te.wait_ge(wload, 80)
            te.wait_ge(dve_s, 1)  # xt9 + c memsets done
            # prologue: full ph1 for tiles 0 and 1
            for tau in range(2):
                te.wait_ge(g_sem[tau % 3], 16)
                for cb in range(NCHUNK):
                    for k in range(KC + 1):
                        ph1_mm(te, tau, cb, k, inc_mm1=(cb == NCHUNK - 1 and k == KC))

            for t in range(nsteps):
                P = (t // TPT) % 2
                c0 = 64 * (t % TPT)
                hpar = t % 2
                if t >= 1:
                    te.wait_ge(rsem[hpar], 16 * ((t + 1) // 2))
                    for cb in range(NCHUNK):
                        for d in range(CORES):
                            mm = te.matmul(
                                ps[P][:, 512 * cb + c0 : 512 * cb + c0 + 64],
                                lhsT=whh_sb[:, G * d + 128 * cb : G * d + 128 * (cb + 1)],
                                rhs=hg[hpar][:, B * d : B * (d + 1)],
                                start=False,
                                stop=(d == CORES - 1),
                                skip_group_check=True,
                            )
                        mm.then_inc(mmr, 1)
                else:
                    # t=0: gates = gx only; bump mmr so ACT waits stay uniform
                    te.nop().then_inc(mmr, 4)
                # interleaved ph1 for tile t//8 + 1 (2..TT-1)
                tf = t // TPT + 1
                if 2 <= tf < TT:
                    s = t % TPT
                    lo, hi = (36 * s) // TPT, (36 * (s + 1)) // TPT
                    for idx in range(lo, hi):
                        cb, k = idx // (KC + 1), idx % (KC + 1)
                        if idx == 0:
                            te.wait_ge(g_sem[tf % 3], 16 * (tf // 3 + 1))
                            # psum parity reuse: tile tf-2 fully consumed
                            # (never blocks in practice; needed for the
                            # happens-before graph)
                            te.wait_ge(mmr, 32 * tf - 32)
                            te.wait_ge(act_s, 32 * tf - 32)
                        ph1_mm(te, tf, cb, k, inc_mm1=(idx == 35))

        # =========== SCALAR (ACT) ===========
        # per step: sig(i), tanh(g), sig(f), sig(o), tanh(c)  [+5]
        @block.scalar
        def _(sc):
            sc.wait_ge(wload, 80)
            for t in range(nsteps):
                P = (t // TPT) % 2
                c0 = 64 * (t % TPT)
                if t == 0:
                    sc.wait_ge(mm1, 1)
                base = 4 * t  # rec(t) gate cb done at mmr = 4t + cb + 1

                def gate_in(cb):
                    return ps[P][:, 512 * cb + c0 : 512 * cb + c0 + 64]

                # sig(i,f): one strided op over banks 0,1
                if_in = ps[P].ap().rearrange("p (cb x) -> p cb x", cb=NCHUNK)[:, 0:2, c0 : c0 + 64]
                sg_if = sg.ap().rearrange("p (cb x) -> p cb x", cb=NCHUNK)[:, 0:2, :]
                if t >= 1:
                    sc.wait_ge(mmr, base + 2)
                sc.activation(sg_if, if_in, AF.Sigmoid).then_inc(act_s, 1)
                if t >= 1:
                    sc.wait_ge(mmr, base + 3)
                sc.activation(sg[:, 2 * B : 3 * B], gate_in(2), AF.Tanh).then_inc(act_s, 1)
                sc.wait_ge(dve_s, 1 + 4 * t + 3)  # c(t) updated
                sc.activation(thc_sb[:, :], c_sb[:, :], AF.Tanh).then_inc(act_s, 1)
                if t >= 1:
                    sc.wait_ge(mmr, base + 4)
                sc.activation(sg[:, 3 * B : 4 * B], gate_in(3), AF.Sigmoid).then_inc(act_s, 1)

        # =========== VECTOR (DVE) ===========
        @block.vector
        def _(ve):
            ve.memset(xt9[:, :], 0.0).then_inc(init_s, 1)
            ve.wait_ge(init_s, 1)
            ve.memset(xt9[0:1, :], 1.0)
            ve.memset(c_sb[:, :], 0.0).then_inc(dve_s, 1)
            for t in range(nsteps):
                ve.wait_ge(act_s, 4 * t + 1)  # sig(i,f)
                ve.wait_ge(dve_s, 4 * t if t >= 1 else 1)  # c_sb; fc_sb read by add(t-1)
                ve.tensor_mul(fc_sb[:, :], sg[:, B : 2 * B], c_sb[:, :]).then_inc(dve_s, 1)
                ve.wait_ge(act_s, 4 * t + 2)  # tanh(g)
                ve.tensor_mul(ig_sb[:, :], sg[:, 0:B], sg[:, 2 * B : 3 * B]).then_inc(dve_s, 1)
                ve.wait_ge(dve_s, 4 * t + 3)  # fc, ig writebacks landed
                if t >= 1:
                    ve.wait_ge(act_s, 4 * t)  # tanh(c) of t-1 read c_sb
                ve.tensor_add(c_sb[:, :], ig_sb[:, :], fc_sb[:, :]).then_inc(dve_s, 1)
                ve.wait_ge(act_s, 4 * t + 4)  # tanh(c), sig(o)
                if t == nsteps - 1:
                    ve.tensor_mul(hout_sb[:, :], sg[:, 3 * B : 4 * B], thc_sb[:, :]).then_inc(dve_s, 1)
                else:
                    if t >= 2:
                        ve.wait_ge(lsem[t % 2], 16 * (t // 2))  # hsrc[t%2] drained
                    ve.tensor_mul(hsrc[t % 2][:, :], sg[:, 3 * B : 4 * B], thc_sb[:, :]).then_inc(dve_s, 1)

    nc.compile()
    return nc


# ---------------------------------------------------------------------------
# host-side input prep
# ---------------------------------------------------------------------------

def prepare_in_maps(source, emb, W_ih, W_hh, b_ih, b_hh, nsteps=S, whh_fp8=False):
    source = np.asarray(source)
    emb = np.asarray(emb, np.float32)
    W_ih = np.asarray(W_ih, np.float32)
    W_hh = np.asarray(W_hh, np.float32)
    b = np.asarray(b_ih, np.float32) + np.asarray(b_hh, np.float32)

    TT = B * nsteps // NT
    emb16 = emb.astype(BF16)

    idx = np.zeros([TT, 128, NIDX], np.int16)
    j = np.arange(NT)
    tprime, bb = j // B, j % B
    for tau in range(TT):
        ids = source[bb, TPT * tau + tprime].astype(np.int16)
        wrapped = ids.reshape(NIDX, 16).T  # [16, NIDX]
        idx[tau] = np.tile(wrapped, (8, 1))

    H = HID
    in_maps = []
    for jc in range(CORES):
        rows = np.concatenate(
            [
                np.arange(CHUNK_TO_BLOCK[cb] * H + 128 * jc,
                          CHUNK_TO_BLOCK[cb] * H + 128 * (jc + 1))
                for cb in range(NCHUNK)
            ]
        )
        Wi = W_ih[rows]           # [512, 1024]
        Wh = W_hh[rows]
        bi = b[rows]              # [512]

        wi4 = Wi.reshape(NCHUNK, 128, KC, 128)           # [cb, m, k, p]
        wih = np.transpose(wi4, (3, 2, 0, 1)).reshape(128, KC * G).astype(BF16)

        wb = np.zeros([128, G], np.float32)
        wb[0] = bi
        wb = wb.astype(BF16)

        # whh: slot d holds logical chunk _M[d]; uniform across cores
        wh4 = Wh.reshape(NCHUNK, 128, KC, 128)           # [cb, m, k, p]
        wh4p = wh4[:, :, _M, :]
        whh = np.transpose(wh4p, (3, 2, 0, 1)).reshape(128, KC * G)
        whh = whh.astype(F8) if whh_fp8 else whh.astype(BF16)

        in_maps.append(
            {
                "emb16": emb16,
                "idx": idx,
                "w_ih": wih,
                "w_b": wb,
                "w_hh": whh,
                "slotoff": np.array([[_M[jc]]], np.int32),
            }
        )
    return in_maps


import os

_BUILD_CACHE = {}
WHH_FP8 = os.environ.get("WHH_FP8", "0") == "1"


def _get_nc(nsteps=S, whh_fp8=WHH_FP8):
    key = (nsteps, whh_fp8)
    if key not in _BUILD_CACHE:
        _BUILD_CACHE[key] = build(nsteps, whh_fp8)
    return _BUILD_CACHE[key]


def kernel(source, emb, W_ih, W_hh, b_ih, b_hh, _trace=False):
    from concourse.bass_utils import run_bass_kernel_spmd

    nc = _get_nc()
    in_maps = prepare_in_maps(source, emb, W_ih, W_hh, b_ih, b_hh, whh_fp8=WHH_FP8)
    res = run_bass_kernel_spmd(nc, in_maps, core_ids=list(range(CORES)), trace=_trace)
    outs = [res.results[i]["out"] for i in range(CORES)]  # each [2, 128, B]
    h = np.concatenate([o[0].T for o in outs], axis=1)  # [B, 8*128]
    c = np.concatenate([o[1].T for o in outs], axis=1)
    out = np.stack([h, c]).astype(np.float32)
    if _trace:
        return out, res
    return out


# ---------------------------------------------------------------------------
# dev: multi-core simulation on a reduced problem
# ---------------------------------------------------------------------------

def _simulate(nsteps=16, whh_fp8=False):
    from concourse import bass_interp, libnrt

    libnrt.get_trn2_nc_mapping.cache_clear()
    libnrt.nc_to_real_nc.cache_clear()
    fake_map = {(d, i): _M[i] for d in range(16) for i in range(8)}
    libnrt.get_trn2_nc_mapping = lambda: fake_map
    libnrt.nc_to_real_nc = lambda dev, i: fake_map[(dev, i)]
    bass_interp.nc_to_real_nc = libnrt.nc_to_real_nc
    bass_interp.pnc_id_to_device_and_real_nc_index = (
        lambda core_id: (core_id // 8, fake_map[(core_id // 8, core_id % 8)])
    )
    fake_rid = {d: d for d in range(16)}
    libnrt.get_device_id_to_routing_id_mapping = lambda: fake_rid
    bass_interp.get_device_id_to_routing_id_mapping = lambda: fake_rid

    rng = np.random.default_rng(0)
    source = rng.integers(0, VOCAB, (B, nsteps)).astype(np.int32)
    emb = rng.standard_normal((VOCAB, EMB), np.float32)
    W_ih = (rng.standard_normal((4 * HID, EMB), np.float32) / np.sqrt(EMB)).astype(np.float32)
    W_hh = (rng.standard_normal((4 * HID, HID), np.float32) / np.sqrt(HID)).astype(np.float32)
    b_ih = np.zeros(4 * HID, np.float32)
    b_hh = np.zeros(4 * HID, np.float32)

    nc = build(nsteps, whh_fp8)
    in_maps = prepare_in_maps(source, emb, W_ih, W_hh, b_ih, b_hh, nsteps, whh_fp8)

    sim = bass_interp.MultiCoreSim(nc, CORES)
    for i in range(CORES):
        for k, v in in_maps[i].items():
            sim.cores[i].tensor(k)[:] = v
    sim.simulate()

    outs = [
        np.array(sim.cores[i].mem_tensor("out")).reshape(2, 128, B)
        for i in range(CORES)
    ]
    h = np.concatenate([o[0].T for o in outs], axis=1)
    c = np.concatenate([o[1].T for o in outs], axis=1)
    actual = np.stack([h, c])

    X = emb[source]
    hh = np.zeros((B, HID), np.float32)
    cc = np.zeros((B, HID), np.float32)
    for t in range(nsteps):
        gates = X[:, t, :] @ W_ih.T + hh @ W_hh.T + b_ih + b_hh
        i_, f_, g_, o_ = np.split(gates, 4, axis=-1)
        i_ = 1 / (1 + np.exp(-i_))
        f_ = 1 / (1 + np.exp(-f_))
        g_ = np.tanh(g_)
        o_ = 1 / (1 + np.exp(-o_))
        cc = f_ * cc + i_ * g_
        hh = o_ * np.tanh(cc)
    expected = np.stack([hh, cc])
    err = np.abs(actual - expected).max() / np.abs(expected).max()
    times = [sim.cores[i].time for i in range(CORES)]
    print(f"sim nsteps={nsteps} fp8={whh_fp8} absmax_rel_err={err:.3e} sim_time_ns={max(times)}")
    return err


if __name__ == "__main__":
    ns = int(sys.argv[1]) if len(sys.argv) > 1 else 16
    fp8 = len(sys.argv) > 2 and sys.argv[2] == "fp8"
    _simulate(ns, fp8)
